# revision 11
# baseline (speedup 1.0000x reference)
"""Trainium2 Bass kernel for the CWLNFace margin-softmax loss head.

Reference computation (B=512, EMB=512, C=70722):
    kernel_norm = kernel / ||kernel||_col            # l2-normalize columns
    cosine      = clip(emb @ kernel_norm, -1+eps, 1-eps)
    out         = S * cos(clip(acos(cosine) - onehot*M*ms, eps, pi-eps))
                  - S * onehot*(M + M*ms)
For every non-label entry the acos/cos round-trip is the identity, so
the dense part is just  S * clip(cosine).  The clip is load-bearing:
embeddings are NOT normalized, so ~1/3 of cosines land outside
[-1+eps, 1-eps] and must saturate.  The margin corrections touch
exactly B=512 entries (one per row) and are applied on the host from
full-precision recomputation.

Device strategy (8 NeuronCores, classnum sharded):
    - Host pre-normalizes kernel columns and folds S into the
      embeddings (S = 64 is a power of two: exact in bf16), so the
      device is a pure matmul + clip/cast pipeline:
          outT[c, b] = clip(dot(emb_b * S, kn_c))
    - Shard C across 8 cores (pad 70722 -> 8*8960 = 71680 with ones).
      Output transposed ([C_sh, B]).
    - Per quad-tile (512 C cols): one 256 KiB input DMA, 16 LDW+MM
      pairs (bf16, N=512, full PE stream rate), one DVE clip pass
      PSUM f32 -> SBUF bf16 (4 banks), one 512 KiB output DMA.  No ACT
      work, no norm matmuls: the PE stream is the critical path and
      all per-iteration DMA/DVE overheads amortize over 4 tiles.
Host reassembles, transposes, and patches the 512 label entries.
"""

import math
import numpy as np

B = 512
EMB = 512
C = 70722
NCORES = 8
CSH = 8960          # per-core padded classnum shard
NT = CSH // 128     # 70 C-tiles of 128 columns
NM = NT // 2        # 35 macro-tiles of 256 columns
S = 64.0
DEV_SCALE = 8.0      # device computes cos*8; host multiplies by 8 (exact)
EPS = 1e-3
MARGIN = 0.4
H = 0.333
CLIP_HI = DEV_SCALE * (1.0 - EPS)  # device-side clip bound

_CACHE = {}


def _build_nc(reps=1):
    from contextlib import ExitStack

    from concourse import bacc, mybir, tile

    f32 = mybir.dt.float32
    bf16 = mybir.dt.bfloat16
    fp8e3 = mybir.dt.float8e3
    OP = mybir.AluOpType

    nc = bacc.Bacc(
        "TRN2",
        target_bir_lowering=False,
        debug=False,
        enable_asserts=False,
    )

    # embT holds (emb * S)^T in bf16 (host-converted; S=64 is exact).
    embT = nc.dram_tensor("embT", [EMB, B], bf16, kind="ExternalInput").ap()
    # Host pre-tiles the pre-normalized kernel shard so each macro-tile is
    # one contiguous 128 KiB block: [macro, partition(EMB%128), sub,
    # chunk(EMB//128), col].
    ksh = nc.dram_tensor(
        "ksh", [NM, 128, 2, 4, 128], bf16, kind="ExternalInput"
    ).ap()
    # Output in e3m4 (host upconverts and multiplies by 8): halves the
    # output DMA stream; the clipped third of entries rounds to exactly
    # 8.0, total rel err 8e-3 vs the 2e-2 gate.
    out = nc.dram_tensor(
        "out", [NM, 128, 2, B], fp8e3, kind="ExternalOutput"
    ).ap()

    with tile.TileContext(nc) as tc, ExitStack() as ctx:
        singles = ctx.enter_context(tc.tile_pool(name="singles", bufs=1))
        kpool = ctx.enter_context(tc.tile_pool(name="k", bufs=6))
        opool = ctx.enter_context(tc.tile_pool(name="o", bufs=4))
        pcpool = ctx.enter_context(tc.tile_pool(name="pc", bufs=2, space="PSUM"))

        # Embeddings^T resident in SBUF: [128, chunk, B], chunk = EMB/128.
        emb_sb = singles.tile([128, 4, B], bf16)
        nc.sync.dma_start(
            out=emb_sb[:], in_=embT.rearrange("(c p) b -> p c b", p=128)
        )

        # Quad iteration: 4 C-tiles (2 DRAM macros) per loop -- one input
        # DMA, 16 LDW+MM, one DVE clip pass, one output DMA.  Halves the
        # per-iteration DMA/DVE instruction overheads vs 2-tile macros.
        NQ = NM // 2  # 17 quads + 1 leftover macro
        for rep in range(reps):
            for q in range(NQ):
                kb_t = kpool.tile([128, 2, 2, 4, 128], bf16)
                # Input halves on the ACT and POOL HWDGE rings (a single
                # 332 GB/s queue cannot keep up with the PE column stream
                # at light load); output stores on the SP ring.
                nc.scalar.dma_start(out=kb_t[:, 0], in_=ksh[2 * q])
                nc.gpsimd.dma_start(out=kb_t[:, 1], in_=ksh[2 * q + 1])
                # Main matmul: cosT_tile = ksh_tile^T @ (S*emb)^T.
                pc = pcpool.tile([128, 4, B], f32)
                for mm in range(2):
                    for u in range(2):
                        for c in range(4):
                            nc.tensor.matmul(
                                pc[:, 2 * mm + u, :],
                                lhsT=kb_t[:, mm, u, c, :],
                                rhs=emb_sb[:, c, :],
                                start=(c == 0),
                                stop=(c == 3),
                            )
                # Clip + cast PSUM f32 -> SBUF bf16 in one DVE pass.
                o_t = opool.tile([128, 2, 2, B], fp8e3)
                nc.vector.tensor_scalar(
                    o_t[:], pc[:], CLIP_HI, -CLIP_HI, OP.min, OP.max
                )
                nc.sync.dma_start(
                    out=out[2 * q : 2 * q + 2].rearrange("m p u b -> p m u b"),
                    in_=o_t[:],
                )
            # Leftover macro 34: half-filled quad-shaped PSUM tile (same
            # pool name, so PSUM stays at one 4-bank allocation x 2 bufs).
            m = NM - 1
            kb2 = kpool.tile([128, 2, 4, 128], bf16, name="kb2")
            nc.scalar.dma_start(out=kb2[:], in_=ksh[m])
            pc2 = pcpool.tile([128, 4, B], f32, name="pc")
            for u in range(2):
                for c in range(4):
                    nc.tensor.matmul(
                        pc2[:, u, :],
                        lhsT=kb2[:, u, c, :],
                        rhs=emb_sb[:, c, :],
                        start=(c == 0),
                        stop=(c == 3),
                    )
            o2 = opool.tile([128, 2, B], fp8e3, name="o2")
            nc.vector.tensor_scalar(
                o2[:], pc2[:, :2, :], CLIP_HI, -CLIP_HI, OP.min, OP.max
            )
            nc.sync.dma_start(out=out[m], in_=o2[:])

    nc.compile()
    return nc


def _get_nc():
    if "nc" not in _CACHE:
        _CACHE["nc"] = _build_nc()
    return _CACHE["nc"]


def make_shards(kfull):
    """Pre-normalize kernel columns, split [EMB, C] into per-core
    macro-major bf16 shards [NM, 128, 2, 4, 128]."""
    import ml_dtypes

    bf16 = np.dtype(ml_dtypes.bfloat16)
    kn = kfull.astype(np.float64)
    kn = kn / np.linalg.norm(kn, axis=0, keepdims=True)
    kn = kn.astype(np.float32)
    shards = []
    for i in range(NCORES):
        lo, hi = i * CSH, (i + 1) * CSH
        if hi <= C:
            shard = kn[:, lo:hi].astype(bf16)
        else:
            shard = np.full((EMB, CSH), 1.0 / math.sqrt(EMB), dtype=bf16)
            shard[:, : C - lo] = kn[:, lo:C].astype(bf16)
        # rows = (chunk, p), cols = (macro, sub, w) -> [macro, p, sub, chunk, w]
        tiled = shard.reshape(4, 128, NM, 2, 128).transpose(2, 1, 3, 0, 4)
        shards.append(np.ascontiguousarray(tiled))
    return shards


def run_device(embbedings, kernel, trace=False):
    """Run the sharded device kernel. Returns (outT [C,B] float32, results)."""
    from concourse.bass_utils import run_bass_kernel_spmd

    nc = _get_nc()

    import ml_dtypes

    embT = np.ascontiguousarray(
        (np.asarray(embbedings, dtype=np.float32).T * np.float32(DEV_SCALE)).astype(
            ml_dtypes.bfloat16
        )
    )
    kfull = np.asarray(kernel, dtype=np.float32)

    in_maps = [{"embT": embT, "ksh": shard} for shard in make_shards(kfull)]

    res = run_bass_kernel_spmd(nc, in_maps, core_ids=list(range(NCORES)), trace=trace)
    # per-core out is [NM, 128, 2, B] macro-major -> row-major [CSH, B]
    parts = [
        np.asarray(r["out"]).transpose(0, 2, 1, 3).reshape(CSH, B)
        for r in res.results
    ]
    outT = np.concatenate(parts, axis=0)[:C].astype(np.float32)  # [C, B]
    outT *= np.float32(S / DEV_SCALE)  # device computed cos*8; want cos*64
    return outT, res


def kernel(embbedings, norms, label, class_sample_num_, kernel):
    outT, _ = run_device(embbedings, kernel)

    # ---- host margin fix-up (touches exactly B entries) ----
    norms = np.asarray(norms, dtype=np.float32)
    csn = np.asarray(class_sample_num_, dtype=np.float32)
    lab = np.asarray(label).astype(np.int64)

    safe = np.clip(norms, 0.001, 100.0)
    safe = safe / (csn[:, None] + 0.001)
    safe = np.clip(safe, 0.001, 100.0).astype(np.float32)
    mean = safe.mean(dtype=np.float64)
    std = safe.std(ddof=1, dtype=np.float64)
    ms = np.clip((safe.astype(np.float64) - mean) / (std + EPS) * H, -1.0, 1.0)[:, 0]

    # Exact label-column cosines on the host (512 length-512 dots): the
    # device's bf16 values would be amplified ~22x by arccos near the clip
    # boundary, so recompute them at full precision.
    rows = np.arange(B)
    emb64 = np.asarray(embbedings, dtype=np.float64)
    cols = np.asarray(kernel, dtype=np.float64)[:, lab]  # [EMB, B]
    dots = np.einsum("be,eb->b", emb64, cols)
    c0 = np.clip(dots / np.linalg.norm(cols, axis=0), -1.0 + EPS, 1.0 - EPS)
    theta = np.arccos(c0) - MARGIN * ms
    theta = np.clip(theta, EPS, math.pi - EPS)
    val = (np.cos(theta) - (MARGIN + MARGIN * ms)) * S
    outT[lab, rows] = val.astype(np.float32)

    return np.ascontiguousarray(outT.T)


# revision 12
# speedup vs baseline: 1.4063x; 1.4063x over previous
"""Trainium2 Bass kernel for the CWLNFace margin-softmax loss head.

Reference computation (B=512, EMB=512, C=70722):
    kernel_norm = kernel / ||kernel||_col            # l2-normalize columns
    cosine      = clip(emb @ kernel_norm, -1+eps, 1-eps)
    out         = S * cos(clip(acos(cosine) - onehot*M*ms, eps, pi-eps))
                  - S * onehot*(M + M*ms)
For every non-label entry the acos/cos round-trip is the identity, so
the dense part is just  S * clip(cosine).  The clip is load-bearing:
embeddings are NOT normalized, so ~1/3 of cosines land outside
[-1+eps, 1-eps] and must saturate.  The margin corrections touch
exactly B=512 entries (one per row) and are applied on the host from
full-precision recomputation.

Device strategy (8 NeuronCores, classnum sharded):
    - Host pre-normalizes kernel columns and folds S into the
      embeddings (S = 64 is a power of two: exact in bf16), so the
      device is a pure matmul + clip/cast pipeline:
          outT[c, b] = clip(dot(emb_b * S, kn_c))
    - Shard C across 8 cores (pad 70722 -> 8*8960 = 71680 with ones).
      Output transposed ([C_sh, B]).
    - Per quad-tile (512 C cols): one 256 KiB input DMA, 16 LDW+MM
      pairs (bf16, N=512, full PE stream rate), one DVE clip pass
      PSUM f32 -> SBUF bf16 (4 banks), one 512 KiB output DMA.  No ACT
      work, no norm matmuls: the PE stream is the critical path and
      all per-iteration DMA/DVE overheads amortize over 4 tiles.
Host reassembles, transposes, and patches the 512 label entries.
"""

import math
import numpy as np

B = 512
EMB = 512
C = 70722
NCORES = 8
CSH = 8960          # per-core padded classnum shard
NT = CSH // 128     # 70 C-tiles of 128 columns
NM = NT // 2        # 35 macro-tiles of 256 columns
S = 64.0
DEV_SCALE = 8.0      # device computes cos*8; host multiplies by 8 (exact)
EPS = 1e-3
MARGIN = 0.4
H = 0.333
CLIP_HI = DEV_SCALE * (1.0 - EPS)  # device-side clip bound

_CACHE = {}


def _build_nc(reps=1):
    from contextlib import ExitStack

    from concourse import bacc, mybir, tile

    f32 = mybir.dt.float32
    bf16 = mybir.dt.bfloat16
    fp8e3 = mybir.dt.float8e3
    OP = mybir.AluOpType

    nc = bacc.Bacc(
        "TRN2",
        target_bir_lowering=False,
        debug=False,
        enable_asserts=False,
    )

    # embT holds (emb * S)^T in bf16 (host-converted; S=64 is exact).
    embT = nc.dram_tensor("embT", [EMB, B], bf16, kind="ExternalInput").ap()
    # Host pre-tiles the pre-normalized kernel shard so each macro-tile is
    # one contiguous 128 KiB block: [macro, partition(EMB%128), sub,
    # chunk(EMB//128), col].
    ksh = nc.dram_tensor(
        "ksh", [NM, 128, 2, 4, 128], bf16, kind="ExternalInput"
    ).ap()
    # Output in e3m4 (host upconverts and multiplies by 8): halves the
    # output DMA stream; the clipped third of entries rounds to exactly
    # 8.0, total rel err 8e-3 vs the 2e-2 gate.
    out = nc.dram_tensor(
        "out", [NM, 128, 2, B], fp8e3, kind="ExternalOutput"
    ).ap()

    with tile.TileContext(nc) as tc, ExitStack() as ctx:
        singles = ctx.enter_context(tc.tile_pool(name="singles", bufs=1))
        kpool = ctx.enter_context(tc.tile_pool(name="k", bufs=6))
        opool = ctx.enter_context(tc.tile_pool(name="o", bufs=4))
        pcpool = ctx.enter_context(tc.tile_pool(name="pc", bufs=2, space="PSUM"))

        # Embeddings^T resident in SBUF: [128, chunk, B], chunk = EMB/128.
        emb_sb = singles.tile([128, 4, B], bf16)
        nc.sync.dma_start(
            out=emb_sb[:], in_=embT.rearrange("(c p) b -> p c b", p=128)
        )

        # Quad iteration: 4 C-tiles (2 DRAM macros) per loop -- one input
        # DMA, 16 LDW+MM, one DVE clip pass, one output DMA.  Halves the
        # per-iteration DMA/DVE instruction overheads vs 2-tile macros.
        NQ = NM // 2  # 17 quads + 1 leftover macro
        for rep in range(reps):
            for q in range(NQ):
                kb_t = kpool.tile([128, 2, 2, 4, 128], bf16)
                # Input loads on the ACT HWDGE ring, output stores on the
                # SP ring.  (POOL-ring input splitting measured 2x slower:
                # software DGE.)
                nc.scalar.dma_start(
                    out=kb_t[:],
                    in_=ksh[2 * q : 2 * q + 2].rearrange(
                        "m p u c w -> p m u c w"
                    ),
                )
                # Main matmul: cosT_tile = ksh_tile^T @ (S*emb)^T.
                pc = pcpool.tile([128, 4, B], f32)
                for mm in range(2):
                    for u in range(2):
                        for c in range(4):
                            nc.tensor.matmul(
                                pc[:, 2 * mm + u, :],
                                lhsT=kb_t[:, mm, u, c, :],
                                rhs=emb_sb[:, c, :],
                                start=(c == 0),
                                stop=(c == 3),
                            )
                # Clip + cast PSUM f32 -> SBUF bf16 in one DVE pass.
                o_t = opool.tile([128, 2, 2, B], fp8e3)
                nc.vector.tensor_scalar(
                    o_t[:], pc[:], CLIP_HI, -CLIP_HI, OP.min, OP.max
                )
                nc.sync.dma_start(
                    out=out[2 * q : 2 * q + 2].rearrange("m p u b -> p m u b"),
                    in_=o_t[:],
                )
            # Leftover macro 34: half-filled quad-shaped PSUM tile (same
            # pool name, so PSUM stays at one 4-bank allocation x 2 bufs).
            m = NM - 1
            kb2 = kpool.tile([128, 2, 4, 128], bf16, name="kb2")
            nc.scalar.dma_start(out=kb2[:], in_=ksh[m])
            pc2 = pcpool.tile([128, 4, B], f32, name="pc")
            for u in range(2):
                for c in range(4):
                    nc.tensor.matmul(
                        pc2[:, u, :],
                        lhsT=kb2[:, u, c, :],
                        rhs=emb_sb[:, c, :],
                        start=(c == 0),
                        stop=(c == 3),
                    )
            o2 = opool.tile([128, 2, B], fp8e3, name="o2")
            nc.vector.tensor_scalar(
                o2[:], pc2[:, :2, :], CLIP_HI, -CLIP_HI, OP.min, OP.max
            )
            nc.sync.dma_start(out=out[m], in_=o2[:])

    nc.compile()
    return nc


def _get_nc():
    if "nc" not in _CACHE:
        _CACHE["nc"] = _build_nc()
    return _CACHE["nc"]


def make_shards(kfull):
    """Pre-normalize kernel columns, split [EMB, C] into per-core
    macro-major bf16 shards [NM, 128, 2, 4, 128]."""
    import ml_dtypes

    bf16 = np.dtype(ml_dtypes.bfloat16)
    kn = kfull.astype(np.float64)
    kn = kn / np.linalg.norm(kn, axis=0, keepdims=True)
    kn = kn.astype(np.float32)
    shards = []
    for i in range(NCORES):
        lo, hi = i * CSH, (i + 1) * CSH
        if hi <= C:
            shard = kn[:, lo:hi].astype(bf16)
        else:
            shard = np.full((EMB, CSH), 1.0 / math.sqrt(EMB), dtype=bf16)
            shard[:, : C - lo] = kn[:, lo:C].astype(bf16)
        # rows = (chunk, p), cols = (macro, sub, w) -> [macro, p, sub, chunk, w]
        tiled = shard.reshape(4, 128, NM, 2, 128).transpose(2, 1, 3, 0, 4)
        shards.append(np.ascontiguousarray(tiled))
    return shards


def run_device(embbedings, kernel, trace=False):
    """Run the sharded device kernel. Returns (outT [C,B] float32, results)."""
    from concourse.bass_utils import run_bass_kernel_spmd

    nc = _get_nc()

    import ml_dtypes

    embT = np.ascontiguousarray(
        (np.asarray(embbedings, dtype=np.float32).T * np.float32(DEV_SCALE)).astype(
            ml_dtypes.bfloat16
        )
    )
    kfull = np.asarray(kernel, dtype=np.float32)

    in_maps = [{"embT": embT, "ksh": shard} for shard in make_shards(kfull)]

    res = run_bass_kernel_spmd(nc, in_maps, core_ids=list(range(NCORES)), trace=trace)
    # per-core out is [NM, 128, 2, B] macro-major -> row-major [CSH, B]
    parts = [
        np.asarray(r["out"]).transpose(0, 2, 1, 3).reshape(CSH, B)
        for r in res.results
    ]
    outT = np.concatenate(parts, axis=0)[:C].astype(np.float32)  # [C, B]
    outT *= np.float32(S / DEV_SCALE)  # device computed cos*8; want cos*64
    return outT, res


def kernel(embbedings, norms, label, class_sample_num_, kernel):
    outT, _ = run_device(embbedings, kernel)

    # ---- host margin fix-up (touches exactly B entries) ----
    norms = np.asarray(norms, dtype=np.float32)
    csn = np.asarray(class_sample_num_, dtype=np.float32)
    lab = np.asarray(label).astype(np.int64)

    safe = np.clip(norms, 0.001, 100.0)
    safe = safe / (csn[:, None] + 0.001)
    safe = np.clip(safe, 0.001, 100.0).astype(np.float32)
    mean = safe.mean(dtype=np.float64)
    std = safe.std(ddof=1, dtype=np.float64)
    ms = np.clip((safe.astype(np.float64) - mean) / (std + EPS) * H, -1.0, 1.0)[:, 0]

    # Exact label-column cosines on the host (512 length-512 dots): the
    # device's bf16 values would be amplified ~22x by arccos near the clip
    # boundary, so recompute them at full precision.
    rows = np.arange(B)
    emb64 = np.asarray(embbedings, dtype=np.float64)
    cols = np.asarray(kernel, dtype=np.float64)[:, lab]  # [EMB, B]
    dots = np.einsum("be,eb->b", emb64, cols)
    c0 = np.clip(dots / np.linalg.norm(cols, axis=0), -1.0 + EPS, 1.0 - EPS)
    theta = np.arccos(c0) - MARGIN * ms
    theta = np.clip(theta, EPS, math.pi - EPS)
    val = (np.cos(theta) - (MARGIN + MARGIN * ms)) * S
    outT[lab, rows] = val.astype(np.float32)

    return np.ascontiguousarray(outT.T)


# revision 13
# speedup vs baseline: 1.4780x; 1.0510x over previous
"""Trainium2 Bass kernel for the CWLNFace margin-softmax loss head.

Reference computation (B=512, EMB=512, C=70722):
    kernel_norm = kernel / ||kernel||_col            # l2-normalize columns
    cosine      = clip(emb @ kernel_norm, -1+eps, 1-eps)
    out         = S * cos(clip(acos(cosine) - onehot*M*ms, eps, pi-eps))
                  - S * onehot*(M + M*ms)
For every non-label entry the acos/cos round-trip is the identity, so
the dense part is just  S * clip(cosine).  The clip is load-bearing:
embeddings are NOT normalized, so ~1/3 of cosines land outside
[-1+eps, 1-eps] and must saturate.  The margin corrections touch
exactly B=512 entries (one per row) and are applied on the host from
full-precision recomputation.

Device strategy (8 NeuronCores, classnum sharded):
    - Host pre-normalizes kernel columns and folds S into the
      embeddings (S = 64 is a power of two: exact in bf16), so the
      device is a pure matmul + clip/cast pipeline:
          outT[c, b] = clip(dot(emb_b * S, kn_c))
    - Shard C across 8 cores (pad 70722 -> 8*8960 = 71680 with ones).
      Output transposed ([C_sh, B]).
    - Per quad-tile (512 C cols): one 256 KiB input DMA, 16 LDW+MM
      pairs (bf16, N=512, full PE stream rate), one DVE clip pass
      PSUM f32 -> SBUF bf16 (4 banks), one 512 KiB output DMA.  No ACT
      work, no norm matmuls: the PE stream is the critical path and
      all per-iteration DMA/DVE overheads amortize over 4 tiles.
Host reassembles, transposes, and patches the 512 label entries.
"""

import math
import numpy as np

B = 512
EMB = 512
C = 70722
NCORES = 8
CSH = 8960          # per-core padded classnum shard
NT = CSH // 128     # 70 C-tiles of 128 columns
NM = NT // 2        # 35 macro-tiles of 256 columns
S = 64.0
EPS = 1e-3
MARGIN = 0.4
H = 0.333
CLIP_HI = S * (1.0 - EPS)

_CACHE = {}


def _build_nc(reps=1):
    from contextlib import ExitStack

    from concourse import bacc, mybir, tile

    f32 = mybir.dt.float32
    bf16 = mybir.dt.bfloat16
    OP = mybir.AluOpType

    nc = bacc.Bacc(
        "TRN2",
        target_bir_lowering=False,
        debug=False,
        enable_asserts=False,
    )

    # embT holds (emb * S)^T in bf16 (host-converted; S=64 is exact).
    embT = nc.dram_tensor("embT", [EMB, B], bf16, kind="ExternalInput").ap()
    # Host pre-tiles the pre-normalized kernel shard so each macro-tile is
    # one contiguous 128 KiB block: [macro, partition(EMB%128), sub,
    # chunk(EMB//128), col].
    ksh = nc.dram_tensor(
        "ksh", [NM, 128, 2, 4, 128], bf16, kind="ExternalInput"
    ).ap()
    out = nc.dram_tensor(
        "out", [NM, 128, 2, B], bf16, kind="ExternalOutput"
    ).ap()

    with tile.TileContext(nc) as tc, ExitStack() as ctx:
        singles = ctx.enter_context(tc.tile_pool(name="singles", bufs=1))
        kpool = ctx.enter_context(tc.tile_pool(name="k", bufs=6))
        opool = ctx.enter_context(tc.tile_pool(name="o", bufs=4))
        pcpool = ctx.enter_context(tc.tile_pool(name="pc", bufs=2, space="PSUM"))

        # Embeddings^T resident in SBUF: [128, chunk, B], chunk = EMB/128.
        emb_sb = singles.tile([128, 4, B], bf16)
        nc.sync.dma_start(
            out=emb_sb[:], in_=embT.rearrange("(c p) b -> p c b", p=128)
        )

        # Quad iteration: 4 C-tiles (2 DRAM macros) per loop -- one input
        # DMA, 16 LDW+MM, one DVE clip pass, one output DMA.  Halves the
        # per-iteration DMA/DVE instruction overheads vs 2-tile macros.
        NQ = NM // 2  # 17 quads + 1 leftover macro
        for rep in range(reps):
            for q in range(NQ):
                kb_t = kpool.tile([128, 2, 2, 4, 128], bf16)
                # Input loads on the ACT HWDGE ring, output stores on the
                # SP ring, so neither sequencer serializes the other's
                # data waits.
                nc.scalar.dma_start(
                    out=kb_t[:],
                    in_=ksh[2 * q : 2 * q + 2].rearrange(
                        "m p u c w -> p m u c w"
                    ),
                )
                # Main matmul: cosT_tile = ksh_tile^T @ (S*emb)^T.
                pc = pcpool.tile([128, 4, B], f32)
                for mm in range(2):
                    for u in range(2):
                        for c in range(4):
                            nc.tensor.matmul(
                                pc[:, 2 * mm + u, :],
                                lhsT=kb_t[:, mm, u, c, :],
                                rhs=emb_sb[:, c, :],
                                start=(c == 0),
                                stop=(c == 3),
                            )
                # Clip + cast PSUM f32 -> SBUF bf16 in one DVE pass.
                o_t = opool.tile([128, 2, 2, B], bf16)
                nc.vector.tensor_scalar(
                    o_t[:], pc[:], CLIP_HI, -CLIP_HI, OP.min, OP.max
                )
                nc.sync.dma_start(
                    out=out[2 * q : 2 * q + 2].rearrange("m p u b -> p m u b"),
                    in_=o_t[:],
                )
            # Leftover macro 34: half-filled quad-shaped PSUM tile (same
            # pool name, so PSUM stays at one 4-bank allocation x 2 bufs).
            m = NM - 1
            kb2 = kpool.tile([128, 2, 4, 128], bf16, name="kb2")
            nc.scalar.dma_start(out=kb2[:], in_=ksh[m])
            pc2 = pcpool.tile([128, 4, B], f32, name="pc")
            for u in range(2):
                for c in range(4):
                    nc.tensor.matmul(
                        pc2[:, u, :],
                        lhsT=kb2[:, u, c, :],
                        rhs=emb_sb[:, c, :],
                        start=(c == 0),
                        stop=(c == 3),
                    )
            o2 = opool.tile([128, 2, B], bf16, name="o2")
            nc.vector.tensor_scalar(
                o2[:], pc2[:, :2, :], CLIP_HI, -CLIP_HI, OP.min, OP.max
            )
            nc.sync.dma_start(out=out[m], in_=o2[:])

    nc.compile()
    return nc


def _get_nc():
    if "nc" not in _CACHE:
        _CACHE["nc"] = _build_nc()
    return _CACHE["nc"]


def make_shards(kfull):
    """Pre-normalize kernel columns, split [EMB, C] into per-core
    macro-major bf16 shards [NM, 128, 2, 4, 128]."""
    import ml_dtypes

    bf16 = np.dtype(ml_dtypes.bfloat16)
    kn = kfull.astype(np.float64)
    kn = kn / np.linalg.norm(kn, axis=0, keepdims=True)
    kn = kn.astype(np.float32)
    shards = []
    for i in range(NCORES):
        lo, hi = i * CSH, (i + 1) * CSH
        if hi <= C:
            shard = kn[:, lo:hi].astype(bf16)
        else:
            shard = np.full((EMB, CSH), 1.0 / math.sqrt(EMB), dtype=bf16)
            shard[:, : C - lo] = kn[:, lo:C].astype(bf16)
        # rows = (chunk, p), cols = (macro, sub, w) -> [macro, p, sub, chunk, w]
        tiled = shard.reshape(4, 128, NM, 2, 128).transpose(2, 1, 3, 0, 4)
        shards.append(np.ascontiguousarray(tiled))
    return shards


def run_device(embbedings, kernel, trace=False):
    """Run the sharded device kernel. Returns (outT [C,B] float32, results)."""
    from concourse.bass_utils import run_bass_kernel_spmd

    nc = _get_nc()

    import ml_dtypes

    embT = np.ascontiguousarray(
        (np.asarray(embbedings, dtype=np.float32).T * np.float32(S)).astype(
            ml_dtypes.bfloat16
        )
    )
    kfull = np.asarray(kernel, dtype=np.float32)

    in_maps = [{"embT": embT, "ksh": shard} for shard in make_shards(kfull)]

    res = run_bass_kernel_spmd(nc, in_maps, core_ids=list(range(NCORES)), trace=trace)
    # per-core out is [NM, 128, 2, B] macro-major -> row-major [CSH, B]
    parts = [
        np.asarray(r["out"]).transpose(0, 2, 1, 3).reshape(CSH, B)
        for r in res.results
    ]
    outT = np.concatenate(parts, axis=0)[:C].astype(np.float32)  # [C, B]
    return outT, res


def kernel(embbedings, norms, label, class_sample_num_, kernel):
    outT, _ = run_device(embbedings, kernel)

    # ---- host margin fix-up (touches exactly B entries) ----
    norms = np.asarray(norms, dtype=np.float32)
    csn = np.asarray(class_sample_num_, dtype=np.float32)
    lab = np.asarray(label).astype(np.int64)

    safe = np.clip(norms, 0.001, 100.0)
    safe = safe / (csn[:, None] + 0.001)
    safe = np.clip(safe, 0.001, 100.0).astype(np.float32)
    mean = safe.mean(dtype=np.float64)
    std = safe.std(ddof=1, dtype=np.float64)
    ms = np.clip((safe.astype(np.float64) - mean) / (std + EPS) * H, -1.0, 1.0)[:, 0]

    # Exact label-column cosines on the host (512 length-512 dots): the
    # device's bf16 values would be amplified ~22x by arccos near the clip
    # boundary, so recompute them at full precision.
    rows = np.arange(B)
    emb64 = np.asarray(embbedings, dtype=np.float64)
    cols = np.asarray(kernel, dtype=np.float64)[:, lab]  # [EMB, B]
    dots = np.einsum("be,eb->b", emb64, cols)
    c0 = np.clip(dots / np.linalg.norm(cols, axis=0), -1.0 + EPS, 1.0 - EPS)
    theta = np.arccos(c0) - MARGIN * ms
    theta = np.clip(theta, EPS, math.pi - EPS)
    val = (np.cos(theta) - (MARGIN + MARGIN * ms)) * S
    outT[lab, rows] = val.astype(np.float32)

    return np.ascontiguousarray(outT.T)
